# revision 4
# baseline (speedup 1.0000x reference)
"""Trainium2 Bass kernel for nn_EBlock (GNN message passing).

Strategy (8 NeuronCores, SPMD single program):
  * Edges are partitioned by DESTINATION node range (host-side sort), so the
    scatter-sum stays core-local -- no AllReduce of [N, HID] partials.
  * Node projection is shard-computed and AllGathered as a f16 gather table.
  * Per-edge gather hv[src] via dma_gather (int16 indices) with the table in
    two halves (lo/hi, int16 range); ONE multi-packet gather per (bin, half)
    round-robined over 4 SWDGE queues -- queues 1-3 generate descriptors
    asynchronously, overlapping queue 0, measured ~2.8ns/idx vs 8ns serial.
  * Edge phase runs in GROUPS of G bins, two passes:
      pass A: y = eT^T @ W_edge (one matmul/tile); a single DVE
              scalar_tensor_tensor copies y to SBUF f16 AND accumulates
              sum(y); a second accumulates sum(y^2).
      group stats: LayerNorm mu/rstd for all G*TPB tiles batched ->
              only 2 ACT table loads per group (Ln+Exp) instead of 2/bin.
      pass B: per-tile DVE prescale (y*rs2+nb), ONE batched Exp per bin,
              msgs = he * gathered, one-hot scatter matmul into PSUM.
  * The segment sum uses the sorted one-hot matmul trick: per 128-dst "bin",
    S[e, w] = (slot[e] == w) and h_bin += msgs_tile^T @ S_tile in PSUM.
"""

import os
import sys

sys.path.insert(0, "/opt/trn_rl_repo")

import numpy as np

import concourse.bass as bass
import concourse.bacc as bacc
import concourse.mybir as mybir
import concourse.tile as tile
from concourse.tile import add_dep_helper
from concourse.bass_utils import run_bass_kernel_spmd

F16 = np.float16

# ---------------------------------------------------------------- config

class Cfg:
    def __init__(self, n_nodes=50000, n_edges=800000, node_in=256, edge_in=64,
                 hid=128, out=16, n_cores=8, lo=32768, eps=1e-5):
        self.N, self.E = n_nodes, n_edges
        self.NODE_IN, self.EDGE_IN, self.HID, self.OUT = node_in, edge_in, hid, out
        self.NC = n_cores
        self.EPS = eps
        self.NPC = (n_nodes + n_cores - 1) // n_cores        # nodes per core
        self.NB = (self.NPC + 127) // 128                     # dst bins per core
        self.NPAD = self.NB * 128                             # padded shard rows
        self.AGROWS = self.NC * self.NPAD                     # allgather table rows
        self.LO = min(lo, self.AGROWS)                        # lo table rows
        self.HIR = self.AGROWS - self.LO                      # hi table rows
        assert self.LO <= 32768 and self.HIR <= 32768
        self.GRPB = int(os.environ.get("K_GRPB", "6"))        # bins per group
        self.NQ = int(os.environ.get("K_NQ", "4"))            # swdge queues
        # K_LO / K_HI / TPB / ETOT set by prep()
        self.K_LO = self.K_HI = self.TPB = self.ETOT = None

    def key(self):
        return (self.N, self.E, self.NODE_IN, self.EDGE_IN, self.HID, self.OUT,
                self.NC, self.LO, self.K_LO, self.K_HI, self.GRPB, self.NQ)


# ---------------------------------------------------------------- host prep

def _to_f16(x):
    return np.asarray(x, dtype=np.float32).astype(F16)


def prep(cfg, node_feats, edge_feats, src, dst,
         W_node, g_node, b_node, W_edge, g_edge, b_edge, W_out, g_out, b_out):
    """Shard/sort/pad the inputs.  Returns (in_maps, meta)."""
    N, E, NC = cfg.N, cfg.E, cfg.NC
    NPC, NB = cfg.NPC, cfg.NB
    HID, EIN, NIN, OUT = cfg.HID, cfg.EDGE_IN, cfg.NODE_IN, cfg.OUT

    src = np.asarray(src).astype(np.int64)
    dst = np.asarray(dst).astype(np.int64)
    node_feats = np.asarray(node_feats, dtype=np.float32)
    edge_feats = np.asarray(edge_feats, dtype=np.float32)

    # position of node i's hv row in the allgathered table
    src_remap = (src // NPC) * cfg.NPAD + (src % NPC)
    is_lo = src_remap < cfg.LO
    core_of_edge = dst // NPC

    percore = []
    for c in range(NC):
        sel = np.nonzero(core_of_edge == c)[0]
        d_loc = (dst[sel] - c * NPC).astype(np.int64)
        lo_cnt = np.bincount(d_loc[is_lo[sel]], minlength=NPC)
        hi_cnt = np.bincount(d_loc[~is_lo[sel]], minlength=NPC)

        # --- bin packing: NB bins of <=128 dst, balancing lo & hi loads
        order = np.argsort(-(lo_cnt + hi_cnt), kind="stable")
        bin_lo = np.zeros(NB); bin_hi = np.zeros(NB)
        bin_n = np.zeros(NB, np.int64)
        assign = np.full(NPC, -1, np.int64)
        slot = np.full(NPC, -1, np.int64)
        t_lo = max(lo_cnt.sum() / NB, 1.0)
        t_hi = max(hi_cnt.sum() / NB, 1.0)
        for d in order:
            cost = np.maximum((bin_lo + lo_cnt[d]) / t_lo,
                              (bin_hi + hi_cnt[d]) / t_hi)
            cost[bin_n >= 128] = np.inf
            b = int(np.argmin(cost))
            assign[d] = b
            slot[d] = bin_n[b]
            bin_n[b] += 1
            bin_lo[b] += lo_cnt[d]
            bin_hi[b] += hi_cnt[d]
        percore.append((sel, d_loc, assign, slot))

    # global tile counts (shared SPMD schedule)
    k_lo = k_hi = 1
    for c in range(NC):
        sel, d_loc, assign, slot = percore[c]
        lo_e = is_lo[sel]
        bin_of_edge = assign[d_loc]
        blc = np.bincount(bin_of_edge[lo_e], minlength=NB)
        bhc = np.bincount(bin_of_edge[~lo_e], minlength=NB)
        k_lo = max(k_lo, int(np.max((blc + 127) // 128)) if blc.size else 1)
        k_hi = max(k_hi, int(np.max((bhc + 127) // 128)) if bhc.size else 1)
    cfg.K_LO, cfg.K_HI = k_lo, k_hi
    cfg.TPB = k_lo + k_hi
    cfg.ETOT = NB * cfg.TPB * 128
    TPB, ETOT = cfg.TPB, cfg.ETOT

    # --- uniformity of gains/biases
    def uni(v):
        v = np.asarray(v, np.float32)
        return (float(v.flat[0]), True) if np.all(v == v.flat[0]) else (0.0, False)
    g_nu, node_g_uni = uni(g_node); b_nu, node_b_uni = uni(b_node)
    g_eu, edge_g_uni = uni(g_edge); b_eu, edge_b_uni = uni(b_edge)
    g_ou, out_g_uni = uni(g_out);  b_ou, out_b_uni = uni(b_out)

    meta = dict(g_nu=g_nu, b_nu=b_nu, g_eu=g_eu, b_eu=b_eu, g_ou=g_ou, b_ou=b_ou,
                node_uni=node_g_uni and node_b_uni,
                edge_uni=edge_g_uni and edge_b_uni,
                out_uni=out_g_uni and out_b_uni,
                inv=[])

    # --- shared weight arrays
    W_node = np.asarray(W_node, np.float32)
    W_edge = np.asarray(W_edge, np.float32)
    W_out = np.asarray(W_out, np.float32)
    assert NIN % 128 == 0
    KN = NIN // 128
    w_node_arr = np.ascontiguousarray(
        W_node.reshape(KN, 128, HID).transpose(1, 0, 2).reshape(128, KN * HID)
    ).astype(F16)
    w_edge_arr = _to_f16(W_edge)
    w_out_arr = _to_f16(W_out)
    iota_arr = np.broadcast_to(
        np.tile(np.arange(128, dtype=np.float32), TPB)[None, :], (128, TPB * 128)
    ).astype(F16)
    g_edge_rep = np.broadcast_to(np.asarray(g_edge, np.float32)[None, :], (128, HID)).copy()
    b_edge_rep = np.broadcast_to(np.asarray(b_edge, np.float32)[None, :], (128, HID)).copy()
    g_node_rep = np.broadcast_to(np.asarray(g_node, np.float32)[None, :], (128, HID)).copy()
    b_node_rep = np.broadcast_to(np.asarray(b_node, np.float32)[None, :], (128, HID)).copy()
    g_out_rep = np.broadcast_to(np.asarray(g_out, np.float32)[None, :], (128, OUT)).copy()
    b_out_rep = np.broadcast_to(np.asarray(b_out, np.float32)[None, :], (128, OUT)).copy()

    in_maps = []
    for c in range(NC):
        sel, d_loc, assign, slot = percore[c]
        lo_e = is_lo[sel]
        bin_of_edge = assign[d_loc]
        slot_of_edge = slot[d_loc]

        # position of each real edge in the padded per-core stream
        ord_e = np.lexsort((src_remap[sel], (~lo_e).astype(np.int64), bin_of_edge))
        sel_o = sel[ord_e]
        bins_o = bin_of_edge[ord_e]
        lo_o = lo_e[ord_e]
        slot_o = slot_of_edge[ord_e]
        # rank within (bin, lo/hi) group
        grp = bins_o * 2 + (~lo_o).astype(np.int64)
        # edges are sorted by grp; rank = index - first index of grp
        first = np.zeros(2 * NB, np.int64)
        cnts = np.bincount(grp, minlength=2 * NB)
        np.cumsum(cnts[:-1], out=first[1:])
        rank = np.arange(len(grp)) - first[grp]
        base = bins_o * (TPB * 128) + np.where(lo_o, 0, k_lo * 128)
        pos = base + rank
        assert len(np.unique(pos)) == len(pos)

        ef_pad = np.zeros((ETOT, EIN), np.float32)
        ef_pad[pos] = edge_feats[sel_o]
        idx_pad = np.zeros(ETOT, np.int64)
        idx_pad[pos] = np.where(lo_o, src_remap[sel_o], src_remap[sel_o] - cfg.LO)
        slot_pad = np.full(ETOT, -1.0, np.float32)
        slot_pad[pos] = slot_o.astype(np.float32)

        edge_T = np.ascontiguousarray(ef_pad.T).astype(F16)
        idx16 = idx_pad.astype(np.int16).reshape(ETOT // 16, 16).T  # [16, ETOT/16]
        src_w = np.ascontiguousarray(np.tile(idx16, (8, 1)))
        dst_sl = np.ascontiguousarray(
            slot_pad.reshape(NB * TPB, 128).T
        ).astype(F16)

        nshard = np.zeros((cfg.NPAD, NIN), np.float32)
        hi = min((c + 1) * NPC, N)
        nshard[: hi - c * NPC] = node_feats[c * NPC: hi]
        node_T = np.ascontiguousarray(nshard.T).astype(F16)

        in_maps.append({
            "edge_T": edge_T, "src_w": src_w, "dst_sl": dst_sl,
            "node_T": node_T, "w_node": w_node_arr, "w_edge": w_edge_arr,
            "w_out": w_out_arr, "iota_in": iota_arr,
            "g_edge_rep": g_edge_rep, "b_edge_rep": b_edge_rep,
            "g_node_rep": g_node_rep, "b_node_rep": b_node_rep,
            "g_out_rep": g_out_rep, "b_out_rep": b_out_rep,
        })

        # output row of local dst d = assign[d]*128 + slot[d]
        real = np.arange(min(NPC, N - c * NPC))
        meta["inv"].append(assign[real] * 128 + slot[real])

    return in_maps, meta


# ---------------------------------------------------------------- device program

def build(cfg, meta):
    NB, TPB, K_LO, K_HI = cfg.NB, cfg.TPB, cfg.K_LO, cfg.K_HI
    HID, EIN, NIN, OUT = cfg.HID, cfg.EDGE_IN, cfg.NODE_IN, cfg.OUT
    ETOT, NPAD, AGROWS, LO = cfg.ETOT, cfg.NPAD, cfg.AGROWS, cfg.LO
    KN = NIN // 128
    EPS = cfg.EPS
    G = cfg.GRPB
    NG = (NB + G - 1) // G
    dt = mybir.dt
    f32, f16, i16 = dt.float32, dt.float16, dt.int16
    AX = mybir.AxisListType
    OP = mybir.AluOpType
    AF = mybir.ActivationFunctionType

    nc = bacc.Bacc("TRN2", target_bir_lowering=False, debug=False,
                   num_devices=cfg.NC, num_swdge_queues=cfg.NQ)

    # register EPS as a usable constant bias AP for nc.scalar.activation
    _t = nc.alloc_sbuf_tensor(f"const-f32-eps", [128, 1], f32)
    nc.gpsimd.memset(_t.ap(), EPS)
    nc.const_aps.aps[(f32, EPS)] = _t.ap()
    nc.all_engine_barrier()

    def din(name, shape, d):
        return nc.dram_tensor(name, shape, d, kind="ExternalInput").ap()

    edge_T = din("edge_T", [EIN, ETOT], f16)
    src_w = din("src_w", [128, ETOT // 16], i16)
    dst_sl = din("dst_sl", [128, NB * TPB], f16)
    node_T = din("node_T", [NIN, NPAD], f16)
    w_node = din("w_node", [128, KN * HID], f16)
    w_edge = din("w_edge", [EIN, HID], f16)
    w_out = din("w_out", [HID, OUT], f16)
    iota_in = din("iota_in", [128, TPB * 128], f16)
    g_edge_rep = din("g_edge_rep", [128, HID], f32)
    b_edge_rep = din("b_edge_rep", [128, HID], f32)
    g_node_rep = din("g_node_rep", [128, HID], f32)
    b_node_rep = din("b_node_rep", [128, HID], f32)
    g_out_rep = din("g_out_rep", [128, OUT], f32)
    b_out_rep = din("b_out_rep", [128, OUT], f32)
    out_ext = nc.dram_tensor("out", [NB * 128, OUT], f32, kind="ExternalOutput").ap()

    hv_in = nc.dram_tensor("hv_in", [NPAD, HID], f16).ap()
    hv_ag = nc.dram_tensor("hv_ag", [AGROWS, HID], f16, addr_space="Shared").ap()
    hv_loc = nc.dram_tensor("hv_loc", [AGROWS, HID], f16).ap()

    g_nu, b_nu = meta["g_nu"], meta["b_nu"]
    g_eu, b_eu = meta["g_eu"], meta["b_eu"]
    g_ou, b_ou = meta["g_ou"], meta["b_ou"]

    with tile.TileContext(nc) as tc:
        cpool = tc.alloc_tile_pool(name="consts", bufs=1)
        ppool = tc.alloc_tile_pool(name="persist", bufs=1)
        spool = tc.alloc_tile_pool(name="stats", bufs=2)
        wkpool = tc.alloc_tile_pool(name="work", bufs=2)
        ygpool = tc.alloc_tile_pool(name="ygrp", bufs=2)
        gpool = tc.alloc_tile_pool(name="gath", bufs=G + 2)
        pspool = tc.alloc_tile_pool(name="ps", bufs=3, space="PSUM")
        hbpool = tc.alloc_tile_pool(name="hb", bufs=2, space="PSUM")

        # ---- constants into SBUF
        wnode_sb = cpool.tile([128, KN, HID], f16)
        nc.sync.dma_start(out=wnode_sb[:], in_=w_node[:])
        wedge_sb = cpool.tile([EIN, HID], f16)
        nc.sync.dma_start(out=wedge_sb[:], in_=w_edge[:])
        wout_sb = cpool.tile([HID, OUT], f16)
        nc.sync.dma_start(out=wout_sb[:], in_=w_out[:])
        iota_sb = cpool.tile([128, TPB, 128], f16)
        nc.sync.dma_start(out=iota_sb[:], in_=iota_in[:])
        srcw_sb = cpool.tile([128, ETOT // 16], i16)
        nc.sync.dma_start(out=srcw_sb[:], in_=src_w[:])
        dst_sb = cpool.tile([128, NB * TPB], f16)
        nc.sync.dma_start(out=dst_sb[:], in_=dst_sl[:])
        if not meta["edge_uni"]:
            ger_sb = cpool.tile([128, HID], f32)
            nc.sync.dma_start(out=ger_sb[:], in_=g_edge_rep[:])
            ber_sb = cpool.tile([128, HID], f32)
            nc.sync.dma_start(out=ber_sb[:], in_=b_edge_rep[:])
        if not meta["node_uni"]:
            gnr_sb = cpool.tile([128, HID], f32)
            nc.sync.dma_start(out=gnr_sb[:], in_=g_node_rep[:])
            bnr_sb = cpool.tile([128, HID], f32)
            nc.sync.dma_start(out=bnr_sb[:], in_=b_node_rep[:])
        if not meta["out_uni"]:
            gor_sb = cpool.tile([128, OUT], f32)
            nc.sync.dma_start(out=gor_sb[:], in_=g_out_rep[:])
            bor_sb = cpool.tile([128, OUT], f32)
            nc.sync.dma_start(out=bor_sb[:], in_=b_out_rep[:])

        # =================================================== phase N: hv
        g_all = ppool.tile([128, NB * HID], f16, tag="g_all")
        ex2_n = spool.tile([128, NB], f32, tag="ex2n")
        sum_n = spool.tile([128, NB], f32, tag="sumn")
        node_r = node_T.rearrange("(a p) m -> p a m", p=128)
        for t in range(NB):
            nt = wkpool.tile([128, KN, 128], f16, tag="nt")
            nc.sync.dma_start(out=nt[:], in_=node_r[:, :, t * 128:(t + 1) * 128])
            ps = pspool.tile([128, HID], f32, tag="mmout")
            for k in range(KN):
                nc.tensor.matmul(ps[:], lhsT=nt[:, k, :], rhs=wnode_sb[:, k, :],
                                 start=(k == 0), stop=(k == KN - 1))
            gsl = g_all[:, t * HID:(t + 1) * HID]
            nc.scalar.activation(out=gsl, in_=ps[:], func=AF.Gelu)
            sqj = wkpool.tile([128, HID], f16, tag="sqj")
            nc.vector.scalar_tensor_tensor(
                out=sqj[:], in0=gsl, scalar=1.0, in1=gsl,
                op0=OP.mult, op1=OP.mult, accum_out=ex2_n[:, t:t + 1])
            nc.vector.reduce_sum(out=sum_n[:, t:t + 1], in_=gsl, axis=AX.X)

        mu_n = spool.tile([128, NB], f32, tag="mun")
        nc.vector.tensor_scalar(out=mu_n[:], in0=sum_n[:], scalar1=1.0 / HID,
                                scalar2=None, op0=OP.mult)
        nc.vector.tensor_scalar(out=ex2_n[:], in0=ex2_n[:], scalar1=1.0 / HID,
                                scalar2=None, op0=OP.mult)
        tmp_n = spool.tile([128, NB], f32, tag="tmpn")
        nc.vector.scalar_tensor_tensor(out=tmp_n[:], in0=mu_n[:], scalar=-1.0,
                                       in1=mu_n[:], op0=OP.mult, op1=OP.mult)
        var_n = spool.tile([128, NB], f32, tag="varn")
        nc.vector.tensor_tensor(out=var_n[:], in0=tmp_n[:], in1=ex2_n[:], op=OP.add)
        lnv_n = spool.tile([128, NB], f32, tag="lnvn")
        nc.scalar.activation(out=lnv_n[:], in_=var_n[:], func=AF.Ln, bias=EPS)
        rstd_n = spool.tile([128, NB], f32, tag="rstdn")
        nc.scalar.activation(out=rstd_n[:], in_=lnv_n[:], func=AF.Exp, scale=-0.5)
        if meta["node_uni"]:
            rs2_n = spool.tile([128, NB], f32, tag="rs2n")
            nc.vector.tensor_scalar(out=rs2_n[:], in0=rstd_n[:], scalar1=g_nu,
                                    scalar2=None, op0=OP.mult)
            nb_n = spool.tile([128, NB], f32, tag="nbn")
            nc.vector.scalar_tensor_tensor(out=nb_n[:], in0=mu_n[:], scalar=-1.0,
                                           in1=rs2_n[:], op0=OP.mult, op1=OP.mult)
            if b_nu != 0.0:
                nc.vector.tensor_scalar(out=nb_n[:], in0=nb_n[:], scalar1=b_nu,
                                        scalar2=None, op0=OP.add)
        for t in range(NB):
            hv_t = wkpool.tile([128, HID], f16, tag="hvt")
            gsl = g_all[:, t * HID:(t + 1) * HID]
            if meta["node_uni"]:
                nc.vector.tensor_scalar(out=hv_t[:], in0=gsl,
                                        scalar1=rs2_n[:, t:t + 1],
                                        scalar2=nb_n[:, t:t + 1],
                                        op0=OP.mult, op1=OP.add)
            else:
                zt = wkpool.tile([128, HID], f32, tag="zt")
                nc.vector.tensor_scalar(out=zt[:], in0=gsl,
                                        scalar1=mu_n[:, t:t + 1],
                                        scalar2=rstd_n[:, t:t + 1],
                                        op0=OP.subtract, op1=OP.mult)
                nc.vector.tensor_tensor(out=zt[:], in0=zt[:], in1=gnr_sb[:], op=OP.mult)
                nc.vector.tensor_tensor(out=hv_t[:], in0=zt[:], in1=bnr_sb[:], op=OP.add)
            nc.sync.dma_start(out=hv_in[t * 128:(t + 1) * 128, :], in_=hv_t[:])

        nc.gpsimd.collective_compute(
            "AllGather", OP.bypass,
            replica_groups=[list(range(cfg.NC))],
            ins=[hv_in[:]], outs=[hv_ag[:]],
        )
        nc.sync.dma_start(out=hv_loc[:], in_=hv_ag[:])

        # =================================================== phase E: edges
        h_sb = ppool.tile([128, NB * 128], f16, tag="h_sb")
        qctr = [0]
        last_exp = None

        def emit_gathers(gb, b):
            """One multi-packet gather per (bin, half), round-robin queues."""
            col0 = b * TPB * 8
            if K_LO > 0:
                nc.gpsimd.dma_gather(
                    out_ap=gb[:, 0:K_LO, :],
                    in_ap=hv_loc[0:LO, :],
                    idxs_ap=srcw_sb[:, col0: col0 + K_LO * 8],
                    num_idxs=K_LO * 128, num_idxs_reg=K_LO * 128,
                    elem_size=HID, single_packet=False,
                    queue_num=qctr[0] % cfg.NQ)
                qctr[0] += 1
            if K_HI > 0:
                nc.gpsimd.dma_gather(
                    out_ap=gb[:, K_LO:TPB, :],
                    in_ap=hv_loc[LO:AGROWS, :],
                    idxs_ap=srcw_sb[:, col0 + K_LO * 8: col0 + TPB * 8],
                    num_idxs=K_HI * 128, num_idxs_reg=K_HI * 128,
                    elem_size=HID, single_packet=False,
                    queue_num=qctr[0] % cfg.NQ)
                qctr[0] += 1

        for g in range(NG):
            bins = list(range(g * G, min((g + 1) * G, NB)))
            Gg = len(bins)
            W = Gg * TPB

            # --- kick off the group's gathers (overlap pass A compute)
            gbs = []
            for b in bins:
                gb = gpool.tile([128, TPB, HID], f16, tag="gb")
                emit_gathers(gb, b)
                gbs.append(gb)

            # --- pass A: y + stats
            y_grp = ygpool.tile([128, G, TPB, HID], f16, tag="ygrp")
            sum_e = spool.tile([128, G * TPB], f32, tag="sume")
            q_e = spool.tile([128, G * TPB], f32, tag="qe")
            for j, b in enumerate(bins):
                eT = wkpool.tile([EIN, TPB * 128], f16, tag="eT")
                nc.sync.dma_start(out=eT[:],
                                  in_=edge_T[:, b * TPB * 128:(b + 1) * TPB * 128])
                for t in range(TPB):
                    hp = pspool.tile([128, HID], f32, tag="mmout")
                    nc.tensor.matmul(hp[:], lhsT=eT[:, t * 128:(t + 1) * 128],
                                     rhs=wedge_sb[:], start=True, stop=True)
                    c = j * TPB + t
                    # copy y to f16 AND accumulate sum(y) in one DVE pass
                    # (in1 is bypassed but must not be a second PSUM read)
                    nc.vector.scalar_tensor_tensor(
                        out=y_grp[:, j, t, :], in0=hp[:], scalar=1.0,
                        in1=iota_sb[:, 0, :],
                        op0=OP.mult, op1=OP.bypass, accum_out=sum_e[:, c:c + 1])
                    sqd = wkpool.tile([128, HID], f16, tag="sqd")
                    nc.vector.scalar_tensor_tensor(
                        out=sqd[:], in0=y_grp[:, j, t, :], scalar=1.0,
                        in1=y_grp[:, j, t, :],
                        op0=OP.mult, op1=OP.mult, accum_out=q_e[:, c:c + 1])

            # --- batched LayerNorm stats for the whole group
            mu_e = spool.tile([128, G * TPB], f32, tag="mue")
            nc.vector.tensor_scalar(out=mu_e[:, :W], in0=sum_e[:, :W],
                                    scalar1=1.0 / HID, scalar2=None, op0=OP.mult)
            ex2_e = spool.tile([128, G * TPB], f32, tag="ex2e")
            nc.vector.tensor_scalar(out=ex2_e[:, :W], in0=q_e[:, :W],
                                    scalar1=1.0 / HID, scalar2=None, op0=OP.mult)
            tmp_e = spool.tile([128, G * TPB], f32, tag="tmpe")
            nc.vector.scalar_tensor_tensor(out=tmp_e[:, :W], in0=mu_e[:, :W],
                                           scalar=-1.0, in1=mu_e[:, :W],
                                           op0=OP.mult, op1=OP.mult)
            var_e = spool.tile([128, G * TPB], f32, tag="vare")
            nc.vector.tensor_tensor(out=var_e[:, :W], in0=tmp_e[:, :W],
                                    in1=ex2_e[:, :W], op=OP.add)
            lnv_e = spool.tile([128, G * TPB], f32, tag="lnve")
            nc.scalar.activation(out=lnv_e[:, :W], in_=var_e[:, :W],
                                 func=AF.Ln, bias=EPS)
            rstd_e = spool.tile([128, G * TPB], f32, tag="rstde")
            nc.scalar.activation(out=rstd_e[:, :W], in_=lnv_e[:, :W],
                                 func=AF.Exp, scale=-0.5)
            if meta["edge_uni"]:
                rs2_e = spool.tile([128, G * TPB], f32, tag="rs2e")
                nc.vector.tensor_scalar(out=rs2_e[:, :W], in0=rstd_e[:, :W],
                                        scalar1=g_eu, scalar2=None, op0=OP.mult)
                nb_e = spool.tile([128, G * TPB], f32, tag="nbe")
                nc.vector.scalar_tensor_tensor(out=nb_e[:, :W], in0=mu_e[:, :W],
                                               scalar=-1.0, in1=rs2_e[:, :W],
                                               op0=OP.mult, op1=OP.mult)
                if b_eu != 0.0:
                    nc.vector.tensor_scalar(out=nb_e[:, :W], in0=nb_e[:, :W],
                                            scalar1=b_eu, scalar2=None, op0=OP.add)

            # --- pass B: exp, msgs, scatter
            for j, b in enumerate(bins):
                ys = wkpool.tile([128, TPB, HID], f16, tag="ys")
                for t in range(TPB):
                    c = j * TPB + t
                    if meta["edge_uni"]:
                        nc.vector.tensor_scalar(out=ys[:, t, :],
                                                in0=y_grp[:, j, t, :],
                                                scalar1=rs2_e[:, c:c + 1],
                                                scalar2=nb_e[:, c:c + 1],
                                                op0=OP.mult, op1=OP.add)
                    else:
                        zt = wkpool.tile([128, HID], f32, tag="zte")
                        nc.vector.tensor_scalar(out=zt[:], in0=y_grp[:, j, t, :],
                                                scalar1=mu_e[:, c:c + 1],
                                                scalar2=rstd_e[:, c:c + 1],
                                                op0=OP.subtract, op1=OP.mult)
                        nc.vector.tensor_tensor(out=zt[:], in0=zt[:], in1=ger_sb[:],
                                                op=OP.mult)
                        nc.vector.tensor_tensor(out=ys[:, t, :], in0=zt[:],
                                                in1=ber_sb[:], op=OP.add)
                he = wkpool.tile([128, TPB, HID], f16, tag="he")
                last_exp = nc.scalar.activation(out=he[:], in_=ys[:], func=AF.Exp)

                msgs = wkpool.tile([128, TPB, HID], f16, tag="msgs")
                nc.vector.tensor_tensor(out=msgs[:], in0=he[:], in1=gbs[j][:],
                                        op=OP.mult)
                S = wkpool.tile([128, TPB, 128], f16, tag="S")
                dsl = dst_sb[:, b * TPB:(b + 1) * TPB, None].to_broadcast(
                    [128, TPB, 128])
                nc.vector.tensor_tensor(out=S[:], in0=iota_sb[:], in1=dsl,
                                        op=OP.is_equal)

                hb = hbpool.tile([128, 128], f32, tag="hb")
                for t in range(TPB):
                    nc.tensor.matmul(hb[:], lhsT=msgs[:, t, :], rhs=S[:, t, :],
                                     start=(t == 0), stop=(t == TPB - 1))
                nc.vector.tensor_copy(out=h_sb[:, b * 128:(b + 1) * 128], in_=hb[:])

        # =================================================== phase OUT
        go_all = ppool.tile([128, NB * OUT], f32, tag="go_all")
        ex2_o = spool.tile([128, NB], f32, tag="ex2o")
        sum_o = spool.tile([128, NB], f32, tag="sumo")
        for b in range(NB):
            op_ps = pspool.tile([128, HID], f32, tag="mmout")   # use [:, :OUT]
            nc.tensor.matmul(op_ps[:, :OUT], lhsT=h_sb[:, b * 128:(b + 1) * 128],
                             rhs=wout_sb[:], start=True, stop=True)
            osl = go_all[:, b * OUT:(b + 1) * OUT]
            gelu_i = nc.scalar.activation(out=osl, in_=op_ps[:, :OUT], func=AF.Gelu)
            if last_exp is not None and b == 0:
                add_dep_helper(gelu_i.ins, last_exp.ins, sync=False,
                               reason="keep OUT-phase gelu after edge-phase exp (ACT tables)")
            sqo = wkpool.tile([128, OUT], f16, tag="sqo")
            nc.vector.scalar_tensor_tensor(
                out=sqo[:], in0=osl, scalar=1.0, in1=osl,
                op0=OP.mult, op1=OP.mult, accum_out=ex2_o[:, b:b + 1])
            nc.vector.reduce_sum(out=sum_o[:, b:b + 1], in_=osl, axis=AX.X)

        mu_o = spool.tile([128, NB], f32, tag="muo")
        nc.vector.tensor_scalar(out=mu_o[:], in0=sum_o[:], scalar1=1.0 / OUT,
                                scalar2=None, op0=OP.mult)
        nc.vector.tensor_scalar(out=ex2_o[:], in0=ex2_o[:], scalar1=1.0 / OUT,
                                scalar2=None, op0=OP.mult)
        tmp_o = spool.tile([128, NB], f32, tag="tmpo")
        nc.vector.scalar_tensor_tensor(out=tmp_o[:], in0=mu_o[:], scalar=-1.0,
                                       in1=mu_o[:], op0=OP.mult, op1=OP.mult)
        var_o = spool.tile([128, NB], f32, tag="varo")
        nc.vector.tensor_tensor(out=var_o[:], in0=tmp_o[:], in1=ex2_o[:], op=OP.add)
        lnv_o = spool.tile([128, NB], f32, tag="lnvo")
        nc.scalar.activation(out=lnv_o[:], in_=var_o[:], func=AF.Ln, bias=EPS)
        rstd_o = spool.tile([128, NB], f32, tag="rstdo")
        nc.scalar.activation(out=rstd_o[:], in_=lnv_o[:], func=AF.Exp, scale=-0.5)
        if meta["out_uni"]:
            rs2_o = spool.tile([128, NB], f32, tag="rs2o")
            nc.vector.tensor_scalar(out=rs2_o[:], in0=rstd_o[:], scalar1=g_ou,
                                    scalar2=None, op0=OP.mult)
            nb_o = spool.tile([128, NB], f32, tag="nbo")
            nc.vector.scalar_tensor_tensor(out=nb_o[:], in0=mu_o[:], scalar=-1.0,
                                           in1=rs2_o[:], op0=OP.mult, op1=OP.mult)
            if b_ou != 0.0:
                nc.vector.tensor_scalar(out=nb_o[:], in0=nb_o[:], scalar1=b_ou,
                                        scalar2=None, op0=OP.add)
        out_all = ppool.tile([128, NB, OUT], f32, tag="out_all")
        for b in range(NB):
            osl = go_all[:, b * OUT:(b + 1) * OUT]
            if meta["out_uni"]:
                nc.vector.tensor_scalar(out=out_all[:, b, :], in0=osl,
                                        scalar1=rs2_o[:, b:b + 1],
                                        scalar2=nb_o[:, b:b + 1],
                                        op0=OP.mult, op1=OP.add)
            else:
                zo = wkpool.tile([128, OUT], f32, tag="zo")
                nc.vector.tensor_scalar(out=zo[:], in0=osl,
                                        scalar1=mu_o[:, b:b + 1],
                                        scalar2=rstd_o[:, b:b + 1],
                                        op0=OP.subtract, op1=OP.mult)
                nc.vector.tensor_tensor(out=zo[:], in0=zo[:], in1=gor_sb[:], op=OP.mult)
                nc.vector.tensor_tensor(out=out_all[:, b, :], in0=zo[:], in1=bor_sb[:],
                                        op=OP.add)
        out_r = out_ext.rearrange("(b p) o -> p b o", p=128)
        nc.sync.dma_start(out=out_r[:], in_=out_all[:])

        for p in (hbpool, pspool, gpool, ygpool, wkpool, spool, ppool, cpool):
            p.release()

    nc.compile()
    return nc


# ---------------------------------------------------------------- entry point

_CACHE = {}


def _get_program(cfg, meta):
    key = cfg.key() + (meta["node_uni"], meta["edge_uni"], meta["out_uni"],
                       meta["g_nu"], meta["b_nu"], meta["g_eu"], meta["b_eu"],
                       meta["g_ou"], meta["b_ou"])
    if key not in _CACHE:
        _CACHE[key] = build(cfg, meta)
    return _CACHE[key]


def run(cfg, inputs, trace=False, trace_cores=None):
    in_maps, meta = prep(cfg, **inputs)
    nc = _get_program(cfg, meta)
    res = run_bass_kernel_spmd(nc, in_maps, core_ids=list(range(cfg.NC)),
                               trace=trace, trace_cores=trace_cores)
    out = np.empty((cfg.N, cfg.OUT), np.float32)
    for c in range(cfg.NC):
        oc = res.results[c]["out"]
        lo = c * cfg.NPC
        hi = min((c + 1) * cfg.NPC, cfg.N)
        out[lo:hi] = oc[meta["inv"][c]]
    return out, res


def kernel(node_feats, edge_feats, src, dst,
           W_node, g_node, b_node, W_edge, g_edge, b_edge,
           W_out, g_out, b_out):
    cfg = Cfg(n_nodes=node_feats.shape[0], n_edges=edge_feats.shape[0],
              node_in=node_feats.shape[1], edge_in=edge_feats.shape[1],
              hid=W_node.shape[1], out=W_out.shape[1])
    out, _ = run(cfg, dict(
        node_feats=node_feats, edge_feats=edge_feats, src=src, dst=dst,
        W_node=W_node, g_node=g_node, b_node=b_node,
        W_edge=W_edge, g_edge=g_edge, b_edge=b_edge,
        W_out=W_out, g_out=g_out, b_out=b_out))
    return out


# revision 6
# speedup vs baseline: 1.7227x; 1.7227x over previous
"""Trainium2 Bass kernel for nn_EBlock (GNN message passing).

Strategy (8 NeuronCores, SPMD single program):
  * Edges are partitioned by DESTINATION node range (host-side sort), so the
    scatter-sum stays core-local -- no AllReduce of [N, HID] partials.
  * Node projection is shard-computed and AllGathered as a f16 gather table.
  * Per-edge gather hv[src] via dma_gather (int16 indices) with the table in
    two halves (lo/hi, int16 range); ONE multi-packet gather per (bin, half)
    round-robined over 4 SWDGE queues -- queues 1-3 generate descriptors
    asynchronously, overlapping queue 0, measured ~2.8ns/idx vs 8ns serial.
  * Edge phase runs in GROUPS of G bins, two passes:
      pass A: y = eT^T @ W_edge (one matmul/tile); a single DVE
              scalar_tensor_tensor copies y to SBUF f16 AND accumulates
              sum(y); a second accumulates sum(y^2).
      group stats: LayerNorm mu/rstd for all G*TPB tiles batched ->
              only 2 ACT table loads per group (Ln+Exp) instead of 2/bin.
      pass B: per-tile DVE prescale (y*rs2+nb), ONE batched Exp per bin,
              msgs = he * gathered, one-hot scatter matmul into PSUM.
  * The segment sum uses the sorted one-hot matmul trick: per 128-dst "bin",
    S[e, w] = (slot[e] == w) and h_bin += msgs_tile^T @ S_tile in PSUM.
"""

import os
import sys

sys.path.insert(0, "/opt/trn_rl_repo")

import numpy as np

import concourse.bass as bass
import concourse.bacc as bacc
import concourse.mybir as mybir
import concourse.tile as tile
from concourse.tile import add_dep_helper
from concourse.bass_utils import run_bass_kernel_spmd

F16 = np.float16

# ---------------------------------------------------------------- config

class Cfg:
    def __init__(self, n_nodes=50000, n_edges=800000, node_in=256, edge_in=64,
                 hid=128, out=16, n_cores=8, lo=32768, eps=1e-5):
        self.N, self.E = n_nodes, n_edges
        self.NODE_IN, self.EDGE_IN, self.HID, self.OUT = node_in, edge_in, hid, out
        self.NC = n_cores
        self.EPS = eps
        self.NPC = (n_nodes + n_cores - 1) // n_cores        # nodes per core
        self.NB = (self.NPC + 127) // 128                     # dst bins per core
        self.NPAD = self.NB * 128                             # padded shard rows
        self.AGROWS = self.NC * self.NPAD                     # allgather table rows
        self.LO = min(lo, self.AGROWS)                        # lo table rows
        self.HIR = self.AGROWS - self.LO                      # hi table rows
        assert self.LO <= 32768 and self.HIR <= 32768
        self.GRPB = int(os.environ.get("K_GRPB", "6"))        # bins per group
        self.NQ = int(os.environ.get("K_NQ", "4"))            # swdge queues
        # K_LO / K_HI / TPB / ETOT set by prep()
        self.K_LO = self.K_HI = self.TPB = self.ETOT = None

    def key(self):
        return (self.N, self.E, self.NODE_IN, self.EDGE_IN, self.HID, self.OUT,
                self.NC, self.LO, self.K_LO, self.K_HI, self.GRPB, self.NQ)


# ---------------------------------------------------------------- host prep

def _to_f16(x):
    return np.asarray(x, dtype=np.float32).astype(F16)


def prep(cfg, node_feats, edge_feats, src, dst,
         W_node, g_node, b_node, W_edge, g_edge, b_edge, W_out, g_out, b_out):
    """Shard/sort/pad the inputs.  Returns (in_maps, meta)."""
    N, E, NC = cfg.N, cfg.E, cfg.NC
    NPC, NB = cfg.NPC, cfg.NB
    HID, EIN, NIN, OUT = cfg.HID, cfg.EDGE_IN, cfg.NODE_IN, cfg.OUT

    src = np.asarray(src).astype(np.int64)
    dst = np.asarray(dst).astype(np.int64)
    node_feats = np.asarray(node_feats, dtype=np.float32)
    edge_feats = np.asarray(edge_feats, dtype=np.float32)

    # position of node i's hv row in the allgathered table
    src_remap = (src // NPC) * cfg.NPAD + (src % NPC)
    is_lo = src_remap < cfg.LO
    core_of_edge = dst // NPC

    percore = []
    for c in range(NC):
        sel = np.nonzero(core_of_edge == c)[0]
        d_loc = (dst[sel] - c * NPC).astype(np.int64)
        lo_cnt = np.bincount(d_loc[is_lo[sel]], minlength=NPC)
        hi_cnt = np.bincount(d_loc[~is_lo[sel]], minlength=NPC)

        # --- bin packing: NB bins of <=128 dst, balancing lo & hi loads
        order = np.argsort(-(lo_cnt + hi_cnt), kind="stable")
        bin_lo = np.zeros(NB); bin_hi = np.zeros(NB)
        bin_n = np.zeros(NB, np.int64)
        assign = np.full(NPC, -1, np.int64)
        slot = np.full(NPC, -1, np.int64)
        t_lo = max(lo_cnt.sum() / NB, 1.0)
        t_hi = max(hi_cnt.sum() / NB, 1.0)
        for d in order:
            cost = np.maximum((bin_lo + lo_cnt[d]) / t_lo,
                              (bin_hi + hi_cnt[d]) / t_hi)
            cost[bin_n >= 128] = np.inf
            b = int(np.argmin(cost))
            assign[d] = b
            slot[d] = bin_n[b]
            bin_n[b] += 1
            bin_lo[b] += lo_cnt[d]
            bin_hi[b] += hi_cnt[d]
        percore.append((sel, d_loc, assign, slot))

    # global tile counts (shared SPMD schedule)
    k_lo = k_hi = 1
    for c in range(NC):
        sel, d_loc, assign, slot = percore[c]
        lo_e = is_lo[sel]
        bin_of_edge = assign[d_loc]
        blc = np.bincount(bin_of_edge[lo_e], minlength=NB)
        bhc = np.bincount(bin_of_edge[~lo_e], minlength=NB)
        k_lo = max(k_lo, int(np.max((blc + 127) // 128)) if blc.size else 1)
        k_hi = max(k_hi, int(np.max((bhc + 127) // 128)) if bhc.size else 1)
    cfg.K_LO, cfg.K_HI = k_lo, k_hi
    cfg.TPB = k_lo + k_hi
    cfg.ETOT = NB * cfg.TPB * 128
    TPB, ETOT = cfg.TPB, cfg.ETOT

    # --- uniformity of gains/biases
    def uni(v):
        v = np.asarray(v, np.float32)
        return (float(v.flat[0]), True) if np.all(v == v.flat[0]) else (0.0, False)
    g_nu, node_g_uni = uni(g_node); b_nu, node_b_uni = uni(b_node)
    g_eu, edge_g_uni = uni(g_edge); b_eu, edge_b_uni = uni(b_edge)
    g_ou, out_g_uni = uni(g_out);  b_ou, out_b_uni = uni(b_out)

    meta = dict(g_nu=g_nu, b_nu=b_nu, g_eu=g_eu, b_eu=b_eu, g_ou=g_ou, b_ou=b_ou,
                node_uni=node_g_uni and node_b_uni,
                edge_uni=edge_g_uni and edge_b_uni,
                out_uni=out_g_uni and out_b_uni,
                inv=[])

    # --- shared weight arrays
    W_node = np.asarray(W_node, np.float32)
    W_edge = np.asarray(W_edge, np.float32)
    W_out = np.asarray(W_out, np.float32)
    assert NIN % 128 == 0
    KN = NIN // 128
    w_node_arr = np.ascontiguousarray(
        W_node.reshape(KN, 128, HID).transpose(1, 0, 2).reshape(128, KN * HID)
    ).astype(F16)
    w_edge_arr = _to_f16(W_edge)
    w_out_arr = _to_f16(W_out)
    iota_arr = np.broadcast_to(
        np.tile(np.arange(128, dtype=np.float32), TPB)[None, :], (128, TPB * 128)
    ).astype(F16)
    g_edge_rep = np.broadcast_to(np.asarray(g_edge, np.float32)[None, :], (128, HID)).copy()
    b_edge_rep = np.broadcast_to(np.asarray(b_edge, np.float32)[None, :], (128, HID)).copy()
    g_node_rep = np.broadcast_to(np.asarray(g_node, np.float32)[None, :], (128, HID)).copy()
    b_node_rep = np.broadcast_to(np.asarray(b_node, np.float32)[None, :], (128, HID)).copy()
    g_out_rep = np.broadcast_to(np.asarray(g_out, np.float32)[None, :], (128, OUT)).copy()
    b_out_rep = np.broadcast_to(np.asarray(b_out, np.float32)[None, :], (128, OUT)).copy()

    in_maps = []
    for c in range(NC):
        sel, d_loc, assign, slot = percore[c]
        lo_e = is_lo[sel]
        bin_of_edge = assign[d_loc]
        slot_of_edge = slot[d_loc]

        # position of each real edge in the padded per-core stream
        ord_e = np.lexsort((src_remap[sel], (~lo_e).astype(np.int64), bin_of_edge))
        sel_o = sel[ord_e]
        bins_o = bin_of_edge[ord_e]
        lo_o = lo_e[ord_e]
        slot_o = slot_of_edge[ord_e]
        # rank within (bin, lo/hi) group
        grp = bins_o * 2 + (~lo_o).astype(np.int64)
        # edges are sorted by grp; rank = index - first index of grp
        first = np.zeros(2 * NB, np.int64)
        cnts = np.bincount(grp, minlength=2 * NB)
        np.cumsum(cnts[:-1], out=first[1:])
        rank = np.arange(len(grp)) - first[grp]
        base = bins_o * (TPB * 128) + np.where(lo_o, 0, k_lo * 128)
        pos = base + rank
        assert len(np.unique(pos)) == len(pos)

        ef_pad = np.zeros((ETOT, EIN), np.float32)
        ef_pad[pos] = edge_feats[sel_o]
        idx_pad = np.zeros(ETOT, np.int64)
        idx_pad[pos] = np.where(lo_o, src_remap[sel_o], src_remap[sel_o] - cfg.LO)
        slot_pad = np.full(ETOT, -1.0, np.float32)
        slot_pad[pos] = slot_o.astype(np.float32)

        edge_T = np.ascontiguousarray(ef_pad.T).astype(F16)
        idx16 = idx_pad.astype(np.int16).reshape(ETOT // 16, 16).T  # [16, ETOT/16]
        src_w = np.ascontiguousarray(np.tile(idx16, (8, 1)))
        dst_sl = np.ascontiguousarray(
            slot_pad.reshape(NB * TPB, 128).T
        ).astype(F16)

        nshard = np.zeros((cfg.NPAD, NIN), np.float32)
        hi = min((c + 1) * NPC, N)
        nshard[: hi - c * NPC] = node_feats[c * NPC: hi]
        node_T = np.ascontiguousarray(nshard.T).astype(F16)

        in_maps.append({
            "edge_T": edge_T, "src_w": src_w, "dst_sl": dst_sl,
            "node_T": node_T, "w_node": w_node_arr, "w_edge": w_edge_arr,
            "w_out": w_out_arr, "iota_in": iota_arr,
            "g_edge_rep": g_edge_rep, "b_edge_rep": b_edge_rep,
            "g_node_rep": g_node_rep, "b_node_rep": b_node_rep,
            "g_out_rep": g_out_rep, "b_out_rep": b_out_rep,
        })

        # output row of local dst d = assign[d]*128 + slot[d]
        real = np.arange(min(NPC, N - c * NPC))
        meta["inv"].append(assign[real] * 128 + slot[real])

    return in_maps, meta


# ---------------------------------------------------------------- device program

def build(cfg, meta):
    NB, TPB, K_LO, K_HI = cfg.NB, cfg.TPB, cfg.K_LO, cfg.K_HI
    HID, EIN, NIN, OUT = cfg.HID, cfg.EDGE_IN, cfg.NODE_IN, cfg.OUT
    ETOT, NPAD, AGROWS, LO = cfg.ETOT, cfg.NPAD, cfg.AGROWS, cfg.LO
    KN = NIN // 128
    EPS = cfg.EPS
    G = cfg.GRPB
    NG = (NB + G - 1) // G
    dt = mybir.dt
    f32, f16, i16 = dt.float32, dt.float16, dt.int16
    AX = mybir.AxisListType
    OP = mybir.AluOpType
    AF = mybir.ActivationFunctionType

    nc = bacc.Bacc("TRN2", target_bir_lowering=False, debug=False,
                   num_devices=cfg.NC, num_swdge_queues=cfg.NQ)

    # register EPS as a usable constant bias AP for nc.scalar.activation
    _t = nc.alloc_sbuf_tensor(f"const-f32-eps", [128, 1], f32)
    nc.gpsimd.memset(_t.ap(), EPS)
    nc.const_aps.aps[(f32, EPS)] = _t.ap()
    nc.all_engine_barrier()

    def din(name, shape, d):
        return nc.dram_tensor(name, shape, d, kind="ExternalInput").ap()

    edge_T = din("edge_T", [EIN, ETOT], f16)
    src_w = din("src_w", [128, ETOT // 16], i16)
    dst_sl = din("dst_sl", [128, NB * TPB], f16)
    node_T = din("node_T", [NIN, NPAD], f16)
    w_node = din("w_node", [128, KN * HID], f16)
    w_edge = din("w_edge", [EIN, HID], f16)
    w_out = din("w_out", [HID, OUT], f16)
    iota_in = din("iota_in", [128, TPB * 128], f16)
    g_edge_rep = din("g_edge_rep", [128, HID], f32)
    b_edge_rep = din("b_edge_rep", [128, HID], f32)
    g_node_rep = din("g_node_rep", [128, HID], f32)
    b_node_rep = din("b_node_rep", [128, HID], f32)
    g_out_rep = din("g_out_rep", [128, OUT], f32)
    b_out_rep = din("b_out_rep", [128, OUT], f32)
    out_ext = nc.dram_tensor("out", [NB * 128, OUT], f32, kind="ExternalOutput").ap()

    hv_in = nc.dram_tensor("hv_in", [NPAD, HID], f16).ap()
    hv_ag = nc.dram_tensor("hv_ag", [AGROWS, HID], f16, addr_space="Shared").ap()
    hv_loc = nc.dram_tensor("hv_loc", [AGROWS, HID], f16).ap()

    g_nu, b_nu = meta["g_nu"], meta["b_nu"]
    g_eu, b_eu = meta["g_eu"], meta["b_eu"]
    g_ou, b_ou = meta["g_ou"], meta["b_ou"]

    with tile.TileContext(nc) as tc:
        cpool = tc.alloc_tile_pool(name="consts", bufs=1)
        ppool = tc.alloc_tile_pool(name="persist", bufs=1)
        spool = tc.alloc_tile_pool(name="stats", bufs=2)
        wkpool = tc.alloc_tile_pool(name="work", bufs=2)
        ygpool = tc.alloc_tile_pool(name="ygrp", bufs=2)
        gpool = tc.alloc_tile_pool(name="gath", bufs=G + 2)
        pspool = tc.alloc_tile_pool(name="ps", bufs=2, space="PSUM")
        ypspool = tc.alloc_tile_pool(name="yps", bufs=3, space="PSUM")
        hbpool = tc.alloc_tile_pool(name="hb", bufs=2, space="PSUM")

        # ---- constants into SBUF
        wnode_sb = cpool.tile([128, KN, HID], f16)
        nc.sync.dma_start(out=wnode_sb[:], in_=w_node[:])
        wedge_sb = cpool.tile([EIN, HID], f16)
        nc.sync.dma_start(out=wedge_sb[:], in_=w_edge[:])
        wout_sb = cpool.tile([HID, OUT], f16)
        nc.sync.dma_start(out=wout_sb[:], in_=w_out[:])
        iota_sb = cpool.tile([128, TPB, 128], f16)
        nc.sync.dma_start(out=iota_sb[:], in_=iota_in[:])
        srcw_sb = cpool.tile([128, ETOT // 16], i16)
        nc.sync.dma_start(out=srcw_sb[:], in_=src_w[:])
        dst_sb = cpool.tile([128, NB * TPB], f16)
        nc.sync.dma_start(out=dst_sb[:], in_=dst_sl[:])
        if not meta["edge_uni"]:
            ger_sb = cpool.tile([128, HID], f32)
            nc.sync.dma_start(out=ger_sb[:], in_=g_edge_rep[:])
            ber_sb = cpool.tile([128, HID], f32)
            nc.sync.dma_start(out=ber_sb[:], in_=b_edge_rep[:])
        if not meta["node_uni"]:
            gnr_sb = cpool.tile([128, HID], f32)
            nc.sync.dma_start(out=gnr_sb[:], in_=g_node_rep[:])
            bnr_sb = cpool.tile([128, HID], f32)
            nc.sync.dma_start(out=bnr_sb[:], in_=b_node_rep[:])
        if not meta["out_uni"]:
            gor_sb = cpool.tile([128, OUT], f32)
            nc.sync.dma_start(out=gor_sb[:], in_=g_out_rep[:])
            bor_sb = cpool.tile([128, OUT], f32)
            nc.sync.dma_start(out=bor_sb[:], in_=b_out_rep[:])

        # =================================================== phase N: hv
        g_all = ppool.tile([128, NB * HID], f16, tag="g_all")
        ex2_n = spool.tile([128, NB], f32, tag="ex2n")
        sum_n = spool.tile([128, NB], f32, tag="sumn")
        node_r = node_T.rearrange("(a p) m -> p a m", p=128)
        for t in range(NB):
            nt = wkpool.tile([128, KN, 128], f16, tag="nt")
            nc.sync.dma_start(out=nt[:], in_=node_r[:, :, t * 128:(t + 1) * 128])
            ps = pspool.tile([128, HID], f32, tag="mmout")
            for k in range(KN):
                nc.tensor.matmul(ps[:], lhsT=nt[:, k, :], rhs=wnode_sb[:, k, :],
                                 start=(k == 0), stop=(k == KN - 1))
            gsl = g_all[:, t * HID:(t + 1) * HID]
            nc.scalar.activation(out=gsl, in_=ps[:], func=AF.Gelu)
            sqj = wkpool.tile([128, HID], f16, tag="sqj")
            nc.vector.scalar_tensor_tensor(
                out=sqj[:], in0=gsl, scalar=1.0, in1=gsl,
                op0=OP.mult, op1=OP.mult, accum_out=ex2_n[:, t:t + 1])
            nc.vector.reduce_sum(out=sum_n[:, t:t + 1], in_=gsl, axis=AX.X)

        mu_n = spool.tile([128, NB], f32, tag="mun")
        nc.vector.tensor_scalar(out=mu_n[:], in0=sum_n[:], scalar1=1.0 / HID,
                                scalar2=None, op0=OP.mult)
        nc.vector.tensor_scalar(out=ex2_n[:], in0=ex2_n[:], scalar1=1.0 / HID,
                                scalar2=None, op0=OP.mult)
        tmp_n = spool.tile([128, NB], f32, tag="tmpn")
        nc.vector.scalar_tensor_tensor(out=tmp_n[:], in0=mu_n[:], scalar=-1.0,
                                       in1=mu_n[:], op0=OP.mult, op1=OP.mult)
        var_n = spool.tile([128, NB], f32, tag="varn")
        nc.vector.tensor_tensor(out=var_n[:], in0=tmp_n[:], in1=ex2_n[:], op=OP.add)
        lnv_n = spool.tile([128, NB], f32, tag="lnvn")
        nc.scalar.activation(out=lnv_n[:], in_=var_n[:], func=AF.Ln, bias=EPS)
        rstd_n = spool.tile([128, NB], f32, tag="rstdn")
        nc.scalar.activation(out=rstd_n[:], in_=lnv_n[:], func=AF.Exp, scale=-0.5)
        if meta["node_uni"]:
            rs2_n = spool.tile([128, NB], f32, tag="rs2n")
            nc.vector.tensor_scalar(out=rs2_n[:], in0=rstd_n[:], scalar1=g_nu,
                                    scalar2=None, op0=OP.mult)
            nb_n = spool.tile([128, NB], f32, tag="nbn")
            nc.vector.scalar_tensor_tensor(out=nb_n[:], in0=mu_n[:], scalar=-1.0,
                                           in1=rs2_n[:], op0=OP.mult, op1=OP.mult)
            if b_nu != 0.0:
                nc.vector.tensor_scalar(out=nb_n[:], in0=nb_n[:], scalar1=b_nu,
                                        scalar2=None, op0=OP.add)
        for t in range(NB):
            hv_t = wkpool.tile([128, HID], f16, tag="hvt")
            gsl = g_all[:, t * HID:(t + 1) * HID]
            if meta["node_uni"]:
                nc.vector.tensor_scalar(out=hv_t[:], in0=gsl,
                                        scalar1=rs2_n[:, t:t + 1],
                                        scalar2=nb_n[:, t:t + 1],
                                        op0=OP.mult, op1=OP.add)
            else:
                zt = wkpool.tile([128, HID], f32, tag="zt")
                nc.vector.tensor_scalar(out=zt[:], in0=gsl,
                                        scalar1=mu_n[:, t:t + 1],
                                        scalar2=rstd_n[:, t:t + 1],
                                        op0=OP.subtract, op1=OP.mult)
                nc.vector.tensor_tensor(out=zt[:], in0=zt[:], in1=gnr_sb[:], op=OP.mult)
                nc.vector.tensor_tensor(out=hv_t[:], in0=zt[:], in1=bnr_sb[:], op=OP.add)
            nc.sync.dma_start(out=hv_in[t * 128:(t + 1) * 128, :], in_=hv_t[:])

        nc.gpsimd.collective_compute(
            "AllGather", OP.bypass,
            replica_groups=[list(range(cfg.NC))],
            ins=[hv_in[:]], outs=[hv_ag[:]],
        )
        nc.sync.dma_start(out=hv_loc[:], in_=hv_ag[:])

        # =================================================== phase E: edges
        h_sb = ppool.tile([128, NB * 128], f16, tag="h_sb")
        qctr = [0]
        last_exp = None

        def emit_gathers(gb, b):
            """One multi-packet gather per (bin, half), round-robin queues."""
            col0 = b * TPB * 8
            if K_LO > 0:
                nc.gpsimd.dma_gather(
                    out_ap=gb[:, 0:K_LO, :],
                    in_ap=hv_loc[0:LO, :],
                    idxs_ap=srcw_sb[:, col0: col0 + K_LO * 8],
                    num_idxs=K_LO * 128, num_idxs_reg=K_LO * 128,
                    elem_size=HID, single_packet=False,
                    queue_num=qctr[0] % cfg.NQ)
                qctr[0] += 1
            if K_HI > 0:
                nc.gpsimd.dma_gather(
                    out_ap=gb[:, K_LO:TPB, :],
                    in_ap=hv_loc[LO:AGROWS, :],
                    idxs_ap=srcw_sb[:, col0 + K_LO * 8: col0 + TPB * 8],
                    num_idxs=K_HI * 128, num_idxs_reg=K_HI * 128,
                    elem_size=HID, single_packet=False,
                    queue_num=qctr[0] % cfg.NQ)
                qctr[0] += 1

        for g in range(NG):
            bins = list(range(g * G, min((g + 1) * G, NB)))
            Gg = len(bins)
            W = Gg * TPB

            # --- kick off the group's gathers (overlap pass A compute)
            gbs = []
            for b in bins:
                gb = gpool.tile([128, TPB, HID], f16, tag="gb")
                emit_gathers(gb, b)
                gbs.append(gb)

            # --- pass A: y (PSUM packs of 4) + bn_stats per pack
            y_grp = ygpool.tile([128, G, TPB, HID], f16, tag="ygrp")
            s6 = spool.tile([128, G * TPB, 6], f32, tag="s6")
            for j, b in enumerate(bins):
                eT = wkpool.tile([EIN, TPB * 128], f16, tag="eT")
                nc.sync.dma_start(out=eT[:],
                                  in_=edge_T[:, b * TPB * 128:(b + 1) * TPB * 128])
                for t0 in range(0, TPB, 4):
                    q = min(4, TPB - t0)
                    y_ps = ypspool.tile([128, 4, HID], f32, tag="yps")
                    for k in range(q):
                        nc.tensor.matmul(y_ps[:, k, :],
                                         lhsT=eT[:, (t0 + k) * 128:(t0 + k + 1) * 128],
                                         rhs=wedge_sb[:], start=True, stop=True)
                    c = j * TPB + t0
                    nc.vector.tensor_copy(out=y_grp[:, j, t0:t0 + q, :],
                                          in_=y_ps[:, :q, :])
                    for k in range(q):
                        nc.vector.bn_stats(out=s6[:, c + k, :],
                                           in_=y_grp[:, j, t0 + k, :])

            # --- batched LayerNorm stats from bn_stats 6-tuples:
            # even/odd halves (count=HID/2 each): mu = (me+mo)/2,
            # var = (M2e+M2o)/HID + ((me-mo)/2)^2
            msum = spool.tile([128, G * TPB], f32, tag="msum")
            nc.vector.tensor_tensor(out=msum[:, :W], in0=s6[:, :W, 1],
                                    in1=s6[:, :W, 4], op=OP.add)
            dm = spool.tile([128, G * TPB], f32, tag="dm")
            nc.vector.tensor_tensor(out=dm[:, :W], in0=s6[:, :W, 1],
                                    in1=s6[:, :W, 4], op=OP.subtract)
            d2 = spool.tile([128, G * TPB], f32, tag="d2")
            nc.vector.scalar_tensor_tensor(out=d2[:, :W], in0=dm[:, :W],
                                           scalar=0.25, in1=dm[:, :W],
                                           op0=OP.mult, op1=OP.mult)
            m2s = spool.tile([128, G * TPB], f32, tag="m2s")
            nc.vector.tensor_tensor(out=m2s[:, :W], in0=s6[:, :W, 2],
                                    in1=s6[:, :W, 5], op=OP.add)
            var_e = spool.tile([128, G * TPB], f32, tag="vare")
            nc.vector.scalar_tensor_tensor(out=var_e[:, :W], in0=m2s[:, :W],
                                           scalar=1.0 / HID, in1=d2[:, :W],
                                           op0=OP.mult, op1=OP.add)
            lnv_e = spool.tile([128, G * TPB], f32, tag="lnve")
            nc.scalar.activation(out=lnv_e[:, :W], in_=var_e[:, :W],
                                 func=AF.Ln, bias=EPS)
            rstd_e = spool.tile([128, G * TPB], f32, tag="rstde")
            nc.scalar.activation(out=rstd_e[:, :W], in_=lnv_e[:, :W],
                                 func=AF.Exp, scale=-0.5)
            mu_e = spool.tile([128, G * TPB], f32, tag="mue")
            nc.vector.tensor_scalar(out=mu_e[:, :W], in0=msum[:, :W],
                                    scalar1=0.5, scalar2=None, op0=OP.mult)
            if meta["edge_uni"]:
                rs2_e = spool.tile([128, G * TPB], f32, tag="rs2e")
                nc.vector.tensor_scalar(out=rs2_e[:, :W], in0=rstd_e[:, :W],
                                        scalar1=g_eu, scalar2=None, op0=OP.mult)
                nb_e = spool.tile([128, G * TPB], f32, tag="nbe")
                nc.vector.scalar_tensor_tensor(out=nb_e[:, :W], in0=msum[:, :W],
                                               scalar=-0.5, in1=rs2_e[:, :W],
                                               op0=OP.mult, op1=OP.mult)
                if b_eu != 0.0:
                    nc.vector.tensor_scalar(out=nb_e[:, :W], in0=nb_e[:, :W],
                                            scalar1=b_eu, scalar2=None, op0=OP.add)

            # --- pass B: exp, msgs, scatter
            for j, b in enumerate(bins):
                he = wkpool.tile([128, TPB, HID], f16, tag="he")
                for t in range(TPB):
                    c = j * TPB + t
                    if meta["edge_uni"]:
                        last_exp = nc.scalar.activation(
                            out=he[:, t, :], in_=y_grp[:, j, t, :], func=AF.Exp,
                            scale=rs2_e[:, c:c + 1], bias=nb_e[:, c:c + 1])
                    else:
                        zt = wkpool.tile([128, HID], f32, tag="zte")
                        nc.vector.tensor_scalar(out=zt[:], in0=y_grp[:, j, t, :],
                                                scalar1=mu_e[:, c:c + 1],
                                                scalar2=rstd_e[:, c:c + 1],
                                                op0=OP.subtract, op1=OP.mult)
                        nc.vector.tensor_tensor(out=zt[:], in0=zt[:], in1=ger_sb[:],
                                                op=OP.mult)
                        nc.vector.tensor_tensor(out=zt[:], in0=zt[:],
                                                in1=ber_sb[:], op=OP.add)
                        last_exp = nc.scalar.activation(out=he[:, t, :], in_=zt[:],
                                                        func=AF.Exp)

                msgs = wkpool.tile([128, TPB, HID], f16, tag="msgs")
                nc.vector.tensor_tensor(out=msgs[:], in0=he[:], in1=gbs[j][:],
                                        op=OP.mult)
                S = wkpool.tile([128, TPB, 128], f16, tag="S")
                dsl = dst_sb[:, b * TPB:(b + 1) * TPB, None].to_broadcast(
                    [128, TPB, 128])
                nc.vector.tensor_tensor(out=S[:], in0=iota_sb[:], in1=dsl,
                                        op=OP.is_equal)

                hb = hbpool.tile([128, 128], f32, tag="hb")
                for t in range(TPB):
                    nc.tensor.matmul(hb[:], lhsT=msgs[:, t, :], rhs=S[:, t, :],
                                     start=(t == 0), stop=(t == TPB - 1))
                nc.vector.tensor_copy(out=h_sb[:, b * 128:(b + 1) * 128], in_=hb[:])

        # =================================================== phase OUT
        go_all = ppool.tile([128, NB * OUT], f32, tag="go_all")
        ex2_o = spool.tile([128, NB], f32, tag="ex2o")
        sum_o = spool.tile([128, NB], f32, tag="sumo")
        for b in range(NB):
            op_ps = pspool.tile([128, HID], f32, tag="mmout")   # use [:, :OUT]
            nc.tensor.matmul(op_ps[:, :OUT], lhsT=h_sb[:, b * 128:(b + 1) * 128],
                             rhs=wout_sb[:], start=True, stop=True)
            osl = go_all[:, b * OUT:(b + 1) * OUT]
            gelu_i = nc.scalar.activation(out=osl, in_=op_ps[:, :OUT], func=AF.Gelu)
            if last_exp is not None and b == 0:
                add_dep_helper(gelu_i.ins, last_exp.ins, sync=False,
                               reason="keep OUT-phase gelu after edge-phase exp (ACT tables)")
            sqo = wkpool.tile([128, OUT], f16, tag="sqo")
            nc.vector.scalar_tensor_tensor(
                out=sqo[:], in0=osl, scalar=1.0, in1=osl,
                op0=OP.mult, op1=OP.mult, accum_out=ex2_o[:, b:b + 1])
            nc.vector.reduce_sum(out=sum_o[:, b:b + 1], in_=osl, axis=AX.X)

        mu_o = spool.tile([128, NB], f32, tag="muo")
        nc.vector.tensor_scalar(out=mu_o[:], in0=sum_o[:], scalar1=1.0 / OUT,
                                scalar2=None, op0=OP.mult)
        nc.vector.tensor_scalar(out=ex2_o[:], in0=ex2_o[:], scalar1=1.0 / OUT,
                                scalar2=None, op0=OP.mult)
        tmp_o = spool.tile([128, NB], f32, tag="tmpo")
        nc.vector.scalar_tensor_tensor(out=tmp_o[:], in0=mu_o[:], scalar=-1.0,
                                       in1=mu_o[:], op0=OP.mult, op1=OP.mult)
        var_o = spool.tile([128, NB], f32, tag="varo")
        nc.vector.tensor_tensor(out=var_o[:], in0=tmp_o[:], in1=ex2_o[:], op=OP.add)
        lnv_o = spool.tile([128, NB], f32, tag="lnvo")
        nc.scalar.activation(out=lnv_o[:], in_=var_o[:], func=AF.Ln, bias=EPS)
        rstd_o = spool.tile([128, NB], f32, tag="rstdo")
        nc.scalar.activation(out=rstd_o[:], in_=lnv_o[:], func=AF.Exp, scale=-0.5)
        if meta["out_uni"]:
            rs2_o = spool.tile([128, NB], f32, tag="rs2o")
            nc.vector.tensor_scalar(out=rs2_o[:], in0=rstd_o[:], scalar1=g_ou,
                                    scalar2=None, op0=OP.mult)
            nb_o = spool.tile([128, NB], f32, tag="nbo")
            nc.vector.scalar_tensor_tensor(out=nb_o[:], in0=mu_o[:], scalar=-1.0,
                                           in1=rs2_o[:], op0=OP.mult, op1=OP.mult)
            if b_ou != 0.0:
                nc.vector.tensor_scalar(out=nb_o[:], in0=nb_o[:], scalar1=b_ou,
                                        scalar2=None, op0=OP.add)
        out_all = ppool.tile([128, NB, OUT], f32, tag="out_all")
        for b in range(NB):
            osl = go_all[:, b * OUT:(b + 1) * OUT]
            if meta["out_uni"]:
                nc.vector.tensor_scalar(out=out_all[:, b, :], in0=osl,
                                        scalar1=rs2_o[:, b:b + 1],
                                        scalar2=nb_o[:, b:b + 1],
                                        op0=OP.mult, op1=OP.add)
            else:
                zo = wkpool.tile([128, OUT], f32, tag="zo")
                nc.vector.tensor_scalar(out=zo[:], in0=osl,
                                        scalar1=mu_o[:, b:b + 1],
                                        scalar2=rstd_o[:, b:b + 1],
                                        op0=OP.subtract, op1=OP.mult)
                nc.vector.tensor_tensor(out=zo[:], in0=zo[:], in1=gor_sb[:], op=OP.mult)
                nc.vector.tensor_tensor(out=out_all[:, b, :], in0=zo[:], in1=bor_sb[:],
                                        op=OP.add)
        out_r = out_ext.rearrange("(b p) o -> p b o", p=128)
        nc.sync.dma_start(out=out_r[:], in_=out_all[:])

        for p in (hbpool, ypspool, pspool, gpool, ygpool, wkpool, spool, ppool, cpool):
            p.release()

    nc.compile()
    return nc


# ---------------------------------------------------------------- entry point

_CACHE = {}


def _get_program(cfg, meta):
    key = cfg.key() + (meta["node_uni"], meta["edge_uni"], meta["out_uni"],
                       meta["g_nu"], meta["b_nu"], meta["g_eu"], meta["b_eu"],
                       meta["g_ou"], meta["b_ou"])
    if key not in _CACHE:
        _CACHE[key] = build(cfg, meta)
    return _CACHE[key]


def run(cfg, inputs, trace=False, trace_cores=None):
    in_maps, meta = prep(cfg, **inputs)
    nc = _get_program(cfg, meta)
    res = run_bass_kernel_spmd(nc, in_maps, core_ids=list(range(cfg.NC)),
                               trace=trace, trace_cores=trace_cores)
    out = np.empty((cfg.N, cfg.OUT), np.float32)
    for c in range(cfg.NC):
        oc = res.results[c]["out"]
        lo = c * cfg.NPC
        hi = min((c + 1) * cfg.NPC, cfg.N)
        out[lo:hi] = oc[meta["inv"][c]]
    return out, res


def kernel(node_feats, edge_feats, src, dst,
           W_node, g_node, b_node, W_edge, g_edge, b_edge,
           W_out, g_out, b_out):
    cfg = Cfg(n_nodes=node_feats.shape[0], n_edges=edge_feats.shape[0],
              node_in=node_feats.shape[1], edge_in=edge_feats.shape[1],
              hid=W_node.shape[1], out=W_out.shape[1])
    out, _ = run(cfg, dict(
        node_feats=node_feats, edge_feats=edge_feats, src=src, dst=dst,
        W_node=W_node, g_node=g_node, b_node=b_node,
        W_edge=W_edge, g_edge=g_edge, b_edge=b_edge,
        W_out=W_out, g_out=g_out, b_out=b_out))
    return out
